# revision 12
# baseline (speedup 1.0000x reference)
"""Trainium2 Bass kernel for nn_LCNNConvolution (GNN message passing), v4.

Math:  out[n] = sum_p softplus( gather(X, NS[n,p,:]).flat @ W.T + b ) - 12*ln2
Key transform: W is block-structured over the 8 neighbor slots, so
    x1[n,p,:] = sum_k Y_k[NS[n,p,k]]  with  Y_k = X @ W_k.T  (b baked in slot 7).

Design (vs the 2-bank v1): the Y table stores SITE PAIRS per row - row t
holds, for each slot k, [Y_k[2t], Y_k[2t+1]] contiguously (128 f16 = 256B per
slot). 25088 rows fit int16 gather indices in ONE bank, so the hot loop
issues exactly ONE 256B dma_gather descriptor per (n,p,k) lookup (v1 needed
two banks = 2x descriptors, each half-wasted on a zero-row dummy). num_idxs
is capped at the HW-safe 1024/call. The pair half is chosen on DVE via
copy_predicated with a per-lookup int8 parity mask (free-dim stride-0
broadcast AP; output AP rank-matched via a padded-(k c)-merge). The
pair-interleaved table is produced directly by the phase-1 matmul: lhsT =
pair-interleaved X.T (contraction dim = 2 sites x 64 feats), rhs =
block-diagonal-by-parity weights [128, 1024], so no on-chip transpose or
strided DRAM writes are needed.

Upload minimization (wall time through the axon tunnel is transfer-bound):
X.T is uploaded as a per-core 1/8 shard and AllGathered on-device across the
8 cores; bias is a single row partition_broadcast on-device; the parity mask
is int8; the output is f16. ~2.9 MB/core of inputs per run vs ~33 MB for v1.

Sharding: data-parallel over sites; each of the 8 cores handles 6250 sites
(50 sites per partition, perm-major column order) and computes its own full
pair-interleaved Y table from the AllGathered X (replicated W).
"""

import numpy as np

import concourse.bass as bass
import concourse.bacc as bacc
import concourse.mybir as mybir
import concourse.tile as tile
from concourse.bass_utils import run_bass_kernel_spmd

N_SITES = 50000
NODE_F = 64
N_PERM = 12
N_NEIGH = 8
OUT_F = 64

N_CORES = 8
SITES_PER_CORE = N_SITES // N_CORES            # 6250
SPP = 50                                       # sites per partition (pad 6400)
PAD_SITES_CORE = 128 * SPP                     # 6400

SITES_PAD = 50176                              # 2 * 25088 (table pad)
T_ROWS = SITES_PAD // 2                        # 25088 pair rows
NBLK = T_ROWS // 128                           # 196 phase-1 blocks

GCOLS = 8                                      # gather cols per partition/call
NIDX = 128 * GCOLS                             # 1024
RCOLS = 24                                     # reduce group = 2 sites
N_CHUNKS = SPP * N_PERM // GCOLS               # 75

F32 = mybir.dt.float32
F16 = mybir.dt.float16
I16 = mybir.dt.int16
I8 = mybir.dt.int8


def build_nc():
    nc = bacc.Bacc("TRN2", target_bir_lowering=False, debug=False)

    x2t = nc.dram_tensor("x2t", [16, T_ROWS], F16, kind="ExternalInput").ap()
    rhs2 = nc.dram_tensor("rhs2", [128, 1024], F16, kind="ExternalInput").ap()
    bias = nc.dram_tensor("bias", [1, 1024], F32, kind="ExternalInput").ap()
    idx = nc.dram_tensor(
        "idx", [N_CHUNKS, 16, N_NEIGH, NIDX // 16], I16, kind="ExternalInput"
    ).ap()
    msk = nc.dram_tensor(
        "msk", [N_CHUNKS, 128, N_NEIGH, GCOLS], I8, kind="ExternalInput"
    ).ap()
    out = nc.dram_tensor(
        "out", [128, SPP * OUT_F], F16, kind="ExternalOutput"
    ).ap()

    with tile.TileContext(nc) as tc:
        with (
            tc.tile_pool(name="persist", bufs=1) as persist,
            tc.tile_pool(name="dram", bufs=1, space="DRAM") as dram,
        ):
            ybig = dram.tile([T_ROWS, 1024], F16)
            half_sb = persist.tile([128, 1], F32)
            nc.vector.memset(half_sb[:], 0.5)

            # all-gather the pair-interleaved X.T from per-core shards
            # (Shared-addr-space output = the fast RDH AllGather path)
            x2t_in = dram.tile([16, T_ROWS], F16)
            x2t_full = nc.dram_tensor(
                "x2t_full_sh", [128, T_ROWS], F16, addr_space="Shared"
            ).ap()
            with tc.tile_pool(name="p0", bufs=1) as p0:
                sh_sb = p0.tile([16, T_ROWS], F16)
                nc.sync.dma_start(out=sh_sb[:], in_=x2t[:])
                nc.sync.dma_start(out=x2t_in[:], in_=sh_sb[:])
                nc.gpsimd.collective_compute(
                    "AllGather",
                    mybir.AluOpType.bypass,
                    replica_groups=[list(range(8))],
                    ins=[x2t_in.opt()],
                    outs=[x2t_full],
                )

            # ---------------- phase 1: pair-interleaved Y table
            with (
                tc.tile_pool(name="p1", bufs=1) as p1,
                tc.tile_pool(name="p1y", bufs=3) as p1y,
                tc.tile_pool(name="p1ps", bufs=2, space="PSUM") as p1ps,
            ):
                x2t_sb = p1.tile([128, T_ROWS], F16)
                nc.sync.dma_start(out=x2t_sb[:], in_=x2t_full[:])
                rhs2_sb = p1.tile([128, 1024], F16)
                nc.sync.dma_start(out=rhs2_sb[:], in_=rhs2[:])
                bias_row = p1.tile([1, 1024], F32)
                nc.sync.dma_start(out=bias_row[:], in_=bias[:])
                bias_sb = p1.tile([128, 1024], F32)
                nc.gpsimd.partition_broadcast(
                    out_ap=bias_sb[:], in_ap=bias_row[:]
                )

                for j in range(NBLK):
                    psum = p1ps.tile([128, 1024], F32, space="PSUM", tag="ps")
                    lhsT = x2t_sb[:, j * 128 : (j + 1) * 128]
                    nc.tensor.matmul(
                        out=psum[:, 0:512],
                        lhsT=lhsT,
                        rhs=rhs2_sb[:, 0:512],
                        start=True,
                        stop=True,
                    )
                    nc.tensor.matmul(
                        out=psum[:, 512:1024],
                        lhsT=lhsT,
                        rhs=rhs2_sb[:, 512:1024],
                        start=True,
                        stop=True,
                    )
                    y_sb = p1y.tile([128, 1024], F16, tag="y")
                    nc.vector.tensor_tensor(
                        out=y_sb[:],
                        in0=psum[:],
                        in1=bias_sb[:],
                        op=mybir.AluOpType.add,
                    )
                    nc.sync.dma_start(
                        out=ybig[j * 128 : (j + 1) * 128, :], in_=y_sb[:]
                    )

            # ---------------- phase 2: single-bank pair gather + select
            with (
                tc.tile_pool(name="p2", bufs=2) as p2,
                tc.tile_pool(name="p2s", bufs=2) as p2s,
            ):
                x1 = None
                for j in range(N_CHUNKS):
                    idx_sb = p2.tile([128, N_NEIGH, NIDX // 16], I16, tag="idx")
                    nc.sync.dma_start(
                        out=idx_sb[:],
                        in_=idx[j]
                        .rearrange("(o p) k n -> o p k n", o=1)
                        .to_broadcast([8, 16, N_NEIGH, NIDX // 16]),
                    )
                    m_sb = p2.tile([128, N_NEIGH, GCOLS], I8, tag="m")
                    nc.sync.dma_start(out=m_sb[:], in_=msk[j])

                    g = p2.tile([128, N_NEIGH, GCOLS, 128], F16, tag="g")
                    for k in range(N_NEIGH):
                        nc.gpsimd.dma_gather(
                            out_ap=g[:, k, :, :],
                            in_ap=ybig[:, k * 128 : (k + 1) * 128],
                            idxs_ap=idx_sb[:, k, :],
                            num_idxs=NIDX,
                            num_idxs_reg=NIDX,
                            elem_size=128,
                            elem_step=1024,
                        )
                    # pair-half select (per-chunk sel tile)
                    sel = p2.tile([128, N_NEIGH, GCOLS, 72], F16, tag="sel")
                    sel_out = sel[:, :, :, 0:64].rearrange("p k c f -> p (k c) f")
                    nc.vector.tensor_copy(sel_out, g[:, :, :, 64:128])
                    nc.vector.copy_predicated(
                        sel_out,
                        m_sb[:]
                        .rearrange("p k c -> p (k c)")
                        .rearrange("p (m o) -> p m o", o=1)
                        .to_broadcast([128, N_NEIGH * GCOLS, 64]),
                        g[:, :, :, 0:64],
                    )
                    if j % 3 == 0:
                        x1 = p2s.tile([128, RCOLS, 64], F32, tag="x1")
                    sub = j % 3
                    nc.vector.tensor_reduce(
                        out=x1[:, sub * GCOLS : (sub + 1) * GCOLS, :],
                        in_=sel[:, :, :, 0:64].rearrange("p k c f -> p c f k"),
                        axis=mybir.AxisListType.X,
                        op=mybir.AluOpType.add,
                    )
                    if sub != 2:
                        continue
                    grp = j // 3  # covers sites 2*grp, 2*grp+1 per partition
                    # softplus(x) - ln2 == Ln(0.5*Exp(x) + 0.5)
                    x2 = p2s.tile([128, RCOLS, 64], F32, tag="x2")
                    nc.scalar.activation(
                        out=x2[:], in_=x1[:], func=mybir.ActivationFunctionType.Exp
                    )
                    nc.scalar.activation(
                        out=x2[:],
                        in_=x2[:],
                        func=mybir.ActivationFunctionType.Ln,
                        scale=0.5,
                        bias=half_sb[:],
                    )
                    acc = p2s.tile([128, 2, 64], F16, tag="acc")
                    with nc.allow_low_precision(reason="12-term softplus sum"):
                        nc.vector.tensor_reduce(
                            out=acc[:],
                            in_=x2[:].rearrange("p (s q) f -> p s f q", q=N_PERM),
                            axis=mybir.AxisListType.X,
                            op=mybir.AluOpType.add,
                        )
                    nc.sync.dma_start(
                        out=out[:, grp * 128 : grp * 128 + 128],
                        in_=acc[:].rearrange("p s f -> p (s f)"),
                    )

    nc.compile()
    return nc


def _host_prep(X_sites, X_NSs, W, b):
    X_sites = np.asarray(X_sites, dtype=np.float32)
    X_NSs = np.asarray(X_NSs)
    W = np.asarray(W, dtype=np.float32)
    b = np.asarray(b, dtype=np.float32)

    Xp = np.zeros((SITES_PAD, NODE_F), dtype=np.float16)
    Xp[:N_SITES] = X_sites.astype(np.float16)
    x2t = np.ascontiguousarray(
        Xp.reshape(T_ROWS, 2, NODE_F).transpose(1, 2, 0).reshape(128, T_ROWS)
    )

    Wk = W.reshape(OUT_F, N_NEIGH, NODE_F)  # [o, k, f']
    rhs2 = np.zeros((128, 1024), dtype=np.float16)
    for par in range(2):
        for k in range(N_NEIGH):
            c0 = k * 128 + par * 64
            rhs2[par * 64 : par * 64 + 64, c0 : c0 + 64] = Wk[:, k, :].T.astype(
                np.float16
            )
    bias = np.zeros((1, 1024), dtype=np.float32)
    for par in range(2):
        c0 = 7 * 128 + par * 64
        bias[0, c0 : c0 + 64] = b

    in_maps = []
    for c in range(N_CORES):
        ns = X_NSs[c * SITES_PER_CORE : (c + 1) * SITES_PER_CORE]
        nsp = np.zeros((PAD_SITES_CORE, N_PERM, N_NEIGH), dtype=np.int64)
        nsp[:SITES_PER_CORE] = ns
        sites = nsp.reshape(128, SPP, N_PERM, N_NEIGH)  # [p, s, q, k]
        t = (sites >> 1).astype(np.int16)
        par = (sites & 1).astype(np.int8)
        # global col ordering per partition: (s, q) -> s*12+q, split into
        # chunks of GCOLS; position i = c8*128 + p
        arr = (
            t.reshape(128, SPP * N_PERM, N_NEIGH)
            .transpose(1, 2, 0)  # [col, k, p]
            .reshape(N_CHUNKS, GCOLS, N_NEIGH, 128)
            .transpose(0, 2, 1, 3)  # [chunk, k, c8, p]
            .reshape(N_CHUNKS, N_NEIGH, NIDX)
        )
        idxv = np.ascontiguousarray(
            arr.reshape(N_CHUNKS, N_NEIGH, NIDX // 16, 16).transpose(0, 3, 1, 2)
        )
        mskv = np.ascontiguousarray(
            (1 - par)
            .reshape(128, SPP * N_PERM, N_NEIGH)
            .transpose(1, 2, 0)  # [col, k, p]
            .reshape(N_CHUNKS, GCOLS, N_NEIGH, 128)
            .transpose(0, 3, 2, 1)  # [chunk, p, k, c8]
            .astype(np.int8)
        )
        in_maps.append(
            {
                "x2t": np.ascontiguousarray(x2t[16 * c : 16 * (c + 1)]),
                "rhs2": rhs2,
                "bias": bias,
                "idx": idxv,
                "msk": mskv,
            }
        )
    return in_maps


_NC_CACHE = {}


def _get_nc():
    if "nc" not in _NC_CACHE:
        _NC_CACHE["nc"] = build_nc()
    return _NC_CACHE["nc"]


def _stitch(results):
    full = np.empty((N_SITES, OUT_F), dtype=np.float32)
    for c, r in enumerate(results):
        o = r["out"].astype(np.float32).reshape(PAD_SITES_CORE, OUT_F)[:SITES_PER_CORE]
        full[c * SITES_PER_CORE : (c + 1) * SITES_PER_CORE] = o
    return full


def kernel(X_sites, X_NSs, W, b, _trace=False):
    nc = _get_nc()
    in_maps = _host_prep(X_sites, X_NSs, W, b)
    res = run_bass_kernel_spmd(
        nc, in_maps, core_ids=list(range(N_CORES)), trace=_trace
    )
    full = _stitch(res.results)
    if _trace:
        return full, res
    return full
